# revision 4
# baseline (speedup 1.0000x reference)
"""Trainium2 Bass kernel for MultiLayerMemoryCachingLSTM — layer-split.

Cores 0-3 run LSTM layer 0, cores 4-7 run layer 1; pair (i, i+4) shares batch
shard [4i, 4i+4). Each core streams only ONE Whh per step (halves the PE-bound
recurrence vs. the dual-layer replicated version). h-blocks of DB steps cross
the pair via ncfw pair AllGathers issued inside the Tile context; zx for the
"other" layer is accumulated into zx_d with per-core zero/real weights so the
SPMD program stays uniform:
  A-cores: phase-B computes zx0 = xp@Wih0 (real); per-block pass adds 0.
  B-cores: phase-B computes zeros; per-block pass adds zx1 = h0_blk@Wih1.
Phase G runs on all pair rows on both cores (identical results); the host
takes A-cores' output.
"""

import math

import numpy as np

B, T, D, H = 16, 256, 1024, 1024
SEG = 16
SCALE = 1.0 / math.sqrt(H)
N_CORES = 8
BL = 4                     # batch rows per pair shard
P = 128
KC = D // P                # 8 contraction chunks
NSUB = 4                   # col groups / hidden subsets
SUB = H // NSUB            # 256 hidden units per subset

_COMPILED = {}


def _gate_perm():
    idx = []
    for j in range(NSUB):
        for g in range(4):
            base = g * H + j * SUB
            idx.extend(range(base, base + SUB))
    return np.array(idx)


def build_bass(Tn=T):
    import concourse.bass as bass
    import concourse.bacc as bacc_mod
    import concourse.mybir as mybir
    import concourse.tile as tile
    from concourse.masks import make_identity

    f32 = mybir.dt.float32
    bf16 = mybir.dt.bfloat16
    AF = mybir.ActivationFunctionType
    TT = mybir.AluOpType
    AX = mybir.AxisListType.X

    Sn = Tn // SEG              # cached segment slots
    ROWS = BL * Tn              # 1024
    NM = ROWS // P              # 8 row chunks
    SB2 = Sn * BL               # bank rows (64)
    DB = 32                     # h-block steps per AllGather
    NB = Tn // DB               # number of real blocks
    NPB = NB + 2                # position blocks (B-cores trail 2 blocks)
    TP_ = NPB * DB              # total positions
    CBL = DB * BL               # block columns (128)

    nc = bacc_mod.Bacc(None, target_bir_lowering=False, num_devices=N_CORES)

    def din(name, shape, dt=bf16):
        return nc.declare_dram_parameter(name, list(shape), dt, isOutput=False)

    xT_ext = din("xT", (D, ROWS), f32)
    wipT = din("wipT", (D, D))
    bip = din("bip", (1, D))
    gip = din("gip", (1, D))
    beip = din("beip", (1, D))
    wpreT = din("wpreT", (D, 4 * H))       # A: Wih0; B: zeros
    bzpre = din("bzpre", (1, 4 * H))       # A: bz0;  B: zeros
    whhT = din("whhT", (H, 4 * H))         # A: Whh0; B: Whh1
    wblkT = din("wblkT", (H, 4 * H))       # A: zeros; B: Wih1
    bzblk = din("bzblk", (1, 4 * H))       # A: zeros; B: bz1
    wqT = din("wqT", (H, H))
    wkT = din("wkT", (H, H))
    wvT = din("wvT", (H, H))
    wg1hT = din("wg1hT", (H, H))
    wg1rT = din("wg1rT", (H, H))
    bg1 = din("bg1", (1, H))
    wg2T = din("wg2T", (H, H))
    bg2 = din("bg2", (1, H))
    woT = din("woT", (H, H))
    bo = din("bo", (1, H))
    wfT = din("wfT", (2 * H, H))
    bf_ = din("bf", (1, H))
    gf = din("gf", (1, H))
    bef = din("bef", (1, H))
    i2blk_ext = din("i2blk", (P, BL))                  # block-diag I4, bf16
    amask_ext = din("amask", (ROWS, SB2), f32)         # additive mask
    mmask_ext = din("mmask", (ROWS, SB2), f32)         # multiplicative mask
    rz_ext = din("rz", (ROWS, 1), f32)                 # 0 for t<SEG else 1

    out_ext = nc.declare_dram_parameter("out", [ROWS, H], f32, isOutput=True)
    dbg_ext = nc.declare_dram_parameter("dbg", [2 * H, CBL], f32, isOutput=True)
    dbg0_ext = nc.declare_dram_parameter("dbg0", [2 * H, CBL], f32, isOutput=True)

    zx_d = nc.dram_tensor("zx_d", [TP_, BL, 4 * H], bf16)
    ret_d = nc.dram_tensor("ret_d", [ROWS, H], bf16)
    # pair-AG h blocks: in = my layer's hT block, out = [L0 slab; L1 slab]
    cc_in = [nc.dram_tensor(f"cc_in{ib}", [H, CBL], bf16) for ib in range(NPB)]
    cc_out = [nc.dram_tensor(f"cc_out{ib}", [2 * H, CBL], bf16)
              for ib in range(NPB)]
    groups = [[i, i + 4] for i in range(4)]

    def ln_relu(pool, src, gamma_sb, beta_sb, out_sb, nfree):
        stat = pool.tile([P, 4], f32, tag="ln_stat")
        nc.vector.reduce_sum(stat[:, 0:1], src[:], axis=AX)
        nc.scalar.mul(stat[:, 1:2], stat[:, 0:1], 1.0 / nfree)
        nc.vector.tensor_tensor(src[:], src[:],
                                stat[:, 1:2].to_broadcast((P, nfree)), TT.subtract)
        sq = pool.tile([P, nfree], f32, tag="ln_sq")
        nc.vector.tensor_tensor(sq[:], src[:], src[:], TT.mult)
        nc.vector.reduce_sum(stat[:, 2:3], sq[:], axis=AX)
        nc.vector.tensor_scalar(stat[:, 3:4], stat[:, 2:3], 1.0 / nfree, 1e-5,
                                TT.mult, TT.add)
        nc.scalar.activation(stat[:, 0:1], stat[:, 3:4], AF.Sqrt)
        nc.vector.reciprocal(stat[:, 2:3], stat[:, 0:1])
        nc.vector.tensor_tensor(src[:], src[:],
                                stat[:, 2:3].to_broadcast((P, nfree)), TT.mult)
        nc.vector.tensor_tensor(src[:], src[:], gamma_sb, TT.mult)
        nc.vector.tensor_tensor(src[:], src[:], beta_sb, TT.add)
        nc.vector.tensor_scalar(out_sb, src[:], 0.0, None, TT.max)

    with tile.TileContext(nc) as tc:
        const_pool = tc.tile_pool(name="const", bufs=1)
        const = const_pool.__enter__()
        ident = const.tile([P, P], bf16)
        make_identity(nc, ident)
        i2blk = const.tile([P, BL], bf16)
        nc.sync.dma_start(i2blk[:], i2blk_ext.ap())
        onesP = const.tile([1, P], bf16)
        nc.gpsimd.memset(onesP[:], 1.0)

        def rep_bias(pool_, psum_pool_, bias_row, nfree, dt_out, tag):
            outt = pool_.tile([P, nfree], dt_out, tag=tag, name=f"rep_{tag}")
            for n0 in range(0, nfree, 512):
                w = min(512, nfree - n0)
                ps = psum_pool_.tile([P, 512], f32, tag="tps", name=f"repps_{tag}_{n0}")
                nc.tensor.matmul(ps[:, :w], onesP[:], bias_row[:, n0:n0 + w],
                                 start=True, stop=True)
                nc.vector.tensor_copy(outt[:, n0:n0 + w], ps[:, :w])
            return outt

        # ---------- Phase A: xp = relu(LN(x@Wip+b)) ----------
        with tc.tile_pool(name="pa", bufs=2) as pool, \
             tc.tile_pool(name="pa_ps", bufs=2, space="PSUM") as psum_pool, \
             tc.tile_pool(name="xpT_pool", bufs=1) as xpT_pool, \
             tc.tile_pool(name="pa_w", bufs=1) as wpool:
            xT_sb = wpool.tile([P, KC, ROWS], bf16, tag="xT")
            nc.gpsimd.dma_start(xT_sb[:], xT_ext.ap().rearrange("(k p) r -> p k r", p=P))
            wip_sb = wpool.tile([P, KC, D], bf16, tag="wip")
            nc.sync.dma_start(wip_sb[:], wipT.ap().rearrange("(k p) n -> p k n", p=P))
            bip_sb = wpool.tile([1, D], bf16, tag="bip")
            nc.sync.dma_start(bip_sb[:], bip.ap())
            gip_row = wpool.tile([1, D], bf16, tag="gip")
            nc.sync.dma_start(gip_row[:], gip.ap())
            beip_row = wpool.tile([1, D], bf16, tag="beip")
            nc.sync.dma_start(beip_row[:], beip.ap())
            gip_rep = rep_bias(wpool, psum_pool, gip_row, D, f32, "gip_rep")
            beip_rep = rep_bias(wpool, psum_pool, beip_row, D, f32, "beip_rep")

            xpT_sb = xpT_pool.tile([P, KC, ROWS], bf16, tag="xpT")

            for m in range(NM):
                xp_ps = psum_pool.tile([P, D], f32, tag="xp_ps")
                for n2 in range(2):
                    nc.tensor.matmul(xp_ps[:, n2 * 512:(n2 + 1) * 512], onesP[:],
                                     bip_sb[:, n2 * 512:(n2 + 1) * 512],
                                     start=True, stop=False)
                    for k in range(KC):
                        nc.tensor.matmul(
                            xp_ps[:, n2 * 512:(n2 + 1) * 512],
                            xT_sb[:, k, m * P:(m + 1) * P],
                            wip_sb[:, k, n2 * 512:(n2 + 1) * 512],
                            start=False, stop=(k == KC - 1))
                xpb = pool.tile([P, D], f32, tag="xpb")
                nc.vector.tensor_copy(xpb[:], xp_ps[:])
                xp_sb = pool.tile([P, D], bf16, tag="xp_sb")
                ln_relu(pool, xpb, gip_rep[:], beip_rep[:], xp_sb[:], D)
                for k in range(KC):
                    tps = psum_pool.tile([P, P], bf16, tag="tps")
                    nc.tensor.transpose(tps[:], xp_sb[:, k * P:(k + 1) * P], ident[:])
                    nc.vector.tensor_copy(xpT_sb[:, k, m * P:(m + 1) * P], tps[:])

            # ---------- Phase B: zx_d = xp @ Wpre + bzpre ----------
            bzpre_sb = wpool.tile([1, 4 * H], bf16, tag="bzpre")
            nc.sync.dma_start(bzpre_sb[:], bzpre.ap())
            with tc.tile_pool(name="pb_w", bufs=2) as wpool2:
                for nn in range(8):
                    wk_sb = wpool2.tile([P, KC, 512], bf16, tag="wih_nk")
                    nc.sync.dma_start(
                        wk_sb[:],
                        wpreT.ap().rearrange("(k p) n -> p k n", p=P)[
                            :, :, nn * 512:(nn + 1) * 512])
                    for m in range(NM):
                        zps = psum_pool.tile([P, 512], f32, tag="zps_b")
                        nc.tensor.matmul(zps[:], onesP[:],
                                         bzpre_sb[:, nn * 512:(nn + 1) * 512],
                                         start=True, stop=False)
                        for k in range(KC):
                            nc.tensor.matmul(zps[:], xpT_sb[:, k, m * P:(m + 1) * P],
                                             wk_sb[:, k, :],
                                             start=False, stop=(k == KC - 1))
                        zsb = pool.tile([P, 512], bf16, tag="zsb")
                        nc.vector.tensor_copy(zsb[:], zps[:])
                        nc.sync.dma_start(
                            zx_d.ap().rearrange("t b n -> (t b) n")[
                                m * P:(m + 1) * P, nn * 512:(nn + 1) * 512],
                            zsb[:])

        # zero zx_d tail rows [Tn, TP_) (A-cores' dummy tail + B warmup)
        with tc.tile_pool(name="pz", bufs=1) as pool:
            zt = pool.tile([P, 4 * H], bf16, tag="ztail")
            nc.any.memzero(zt[:])
            ntail = (TP_ - Tn) * BL
            for r0 in range(0, ntail, P):
                nc.sync.dma_start(
                    zx_d.ap().rearrange("t b n -> (t b) n")[
                        Tn * BL + r0:Tn * BL + r0 + P, :], zt[:])

        # ---------- Single-layer recurrence with pair AllGathers ----------
        def fused_lstm():
            CHZ = 2            # zx chunk steps
            CH = 4             # hT/hn chunk steps
            with tc.tile_pool(name="fl", bufs=2) as pool, \
                 tc.tile_pool(name="fl_zx", bufs=2) as zxpool, \
                 tc.tile_pool(name="fl_st", bufs=2) as stpool, \
                 tc.tile_pool(name="fl_ps", bufs=3, space="PSUM") as psum_pool, \
                 tc.tile_pool(name="fl_ps_sm", bufs=2, space="PSUM") as psum_small, \
                 tc.tile_pool(name="fl_w", bufs=1) as wpool, \
                 tc.tile_pool(name="fl_wih", bufs=2) as wihpool, \
                 tc.tile_pool(name="fl_hblk", bufs=2) as hblkpool:
                whh_sb = wpool.tile([P, KC, 4 * H], bf16, tag="whh", name="whh")
                nc.sync.dma_start(whh_sb[:], whhT.ap().rearrange(
                    "(k p) n -> p k n", p=P))
                bzblk_sb = wpool.tile([1, 4 * H], bf16, tag="bzblk")
                nc.sync.dma_start(bzblk_sb[:], bzblk.ap())

                hT0 = wpool.tile([P, 2 * P], bf16, tag="hT0", name="hT0")
                nc.any.memzero(hT0[:])
                c0 = stpool.tile([P, SUB], f32, tag="c", name="c_init")
                nc.any.memzero(c0[:])
                st = dict(hT0=hT0, c=c0, zx=None, zxnext=None,
                          hTbuf=None, hTprev=None, hnbuf=None, hnprev=None)

                def emit_transposes(t):
                    s = st
                    sl = t % CH
                    hn = s["hnbuf"]
                    for half in range(2):
                        tps = psum_small.tile([P, P], bf16, tag="tps",
                                              name=f"tp_{t}_{half}")
                        nc.tensor.transpose(
                            tps[:], hn[:, sl, half * P:(half + 1) * P], ident[:])
                        nc.vector.tensor_copy(
                            s["hTbuf"][:, sl, half * P:(half + 1) * P], tps[:])

                def flush_chunk(t_last):
                    """DMA the completed hT chunk ending at t_last into its
                    cc_in block tensor (columns relative to the block)."""
                    s = st
                    t0 = t_last - CH + 1
                    ib = t0 // DB
                    c0_ = (t0 - ib * DB) * BL
                    for half in range(2):
                        for j in range(NSUB):
                            off = 128 * half + 32 * j
                            nc.sync.dma_start(
                                cc_in[ib].ap()[256 * j + 128 * half:
                                               256 * j + 128 * (half + 1),
                                               c0_:c0_ + CH * BL].rearrange(
                                    "u (t b) -> u t b", b=BL),
                                s["hTbuf"][:, :, off:off + BL])

                def load_zx(t0):
                    zx = zxpool.tile([P, CHZ, H], bf16, tag="zx",
                                     name=f"zx_{t0}")
                    for j in range(NSUB):
                        nc.sync.dma_start(
                            zx[32 * j:32 * j + BL, :, :],
                            zx_d.ap()[t0:t0 + CHZ, :,
                                      j * H:(j + 1) * H].rearrange(
                                "t b n -> b t n"))
                    return zx

                def layer_mm(t):
                    s = st
                    if t % CH == 0:
                        if t > 0:
                            emit_transposes(t - 1)
                            flush_chunk(t - 1)
                        s["hTprev"] = s["hTbuf"]
                        s["hTbuf"] = stpool.tile([P, CH, 2 * P], bf16,
                                                 tag="hTb", name=f"hTb_{t}")
                        s["hnprev"] = s["hnbuf"]
                        s["hnbuf"] = stpool.tile([P, CH, SUB], bf16,
                                                 tag="hnb", name=f"hnb_{t}")
                    elif t > 0:
                        emit_transposes(t - 1)
                    if t % CHZ == 0:
                        s["zx"] = load_zx(t) if t == 0 else s["zxnext"]
                        s["zxnext"] = load_zx(t + CHZ) if t + CHZ < TP_ else None
                    sl = t % CH
                    slz = t % CHZ
                    zps = psum_pool.tile([P, H], f32, tag="zps",
                                         name=f"zps_{t}")
                    for n2 in range(2):
                        for j in range(NSUB):
                            nc.tensor.matmul(
                                zps[32 * j:32 * j + BL,
                                    n2 * 512:(n2 + 1) * 512],
                                i2blk[32 * j:32 * j + BL, :],
                                s["zx"][32 * j:32 * j + BL, slz,
                                        n2 * 512:(n2 + 1) * 512],
                                start=True, stop=False,
                                tile_position=(32 * j, 32 * j))
                    for k in range(KC):
                        half, jj = k % 2, k // 2
                        off = 128 * half + 32 * jj
                        if t == 0:
                            lhs = s["hT0"][:, off:off + BL]
                        elif sl == 0:
                            lhs = s["hTprev"][:, CH - 1, off:off + BL]
                        else:
                            lhs = s["hTbuf"][:, sl - 1, off:off + BL]
                        for n2 in range(2):
                            for j in range(NSUB):
                                nc.tensor.matmul(
                                    zps[32 * j:32 * j + BL,
                                        n2 * 512:(n2 + 1) * 512],
                                    lhs,
                                    whh_sb[:, k, j * H + n2 * 512:
                                           j * H + (n2 + 1) * 512],
                                    start=False, stop=(k == KC - 1),
                                    tile_position=(0, 32 * j))
                    return zps

                def layer_gates(t, zps):
                    s = st
                    sl = t % CH
                    sig = pool.tile([P, H], bf16, tag="sig", name=f"sg_{t}")
                    nc.scalar.activation(sig[:], zps[:], AF.Sigmoid)
                    tg = pool.tile([P, SUB], bf16, tag="tg", name=f"tg_{t}")
                    nc.scalar.activation(tg[:], zps[:, 2 * SUB:3 * SUB], AF.Tanh)
                    cf = pool.tile([P, SUB], f32, tag="cf", name=f"cf_{t}")
                    nc.vector.tensor_tensor(cf[:], sig[:, SUB:2 * SUB],
                                            s["c"][:], TT.mult)
                    ig = pool.tile([P, SUB], f32, tag="ig", name=f"ig_{t}")
                    nc.vector.tensor_tensor(ig[:], sig[:, 0:SUB], tg[:], TT.mult)
                    c_new = stpool.tile([P, SUB], f32, tag="c", name=f"c_{t}")
                    nc.vector.tensor_tensor(c_new[:], cf[:], ig[:], TT.add)
                    s["c"] = c_new
                    tct = pool.tile([P, SUB], bf16, tag="tct", name=f"tc_{t}")
                    nc.scalar.activation(tct[:], c_new[:], AF.Tanh)
                    nc.vector.tensor_tensor(s["hnbuf"][:, sl, :],
                                            sig[:, 3 * SUB:4 * SUB], tct[:],
                                            TT.mult)

                hblk_live = {}

                def zxblk_hblk(bb):
                    """Load the L0 slab of block bb-DB..bb from cc_out."""
                    ib = bb // DB - 1
                    hblk = hblkpool.tile([P, KC, CBL], bf16, tag="hblk",
                                         name=f"hblk_{bb}")
                    nc.scalar.dma_start(
                        hblk[:], cc_out[ib].ap()[0:H, :].rearrange(
                            "(k p) r -> p k r", p=P))
                    hblk_live[bb] = hblk

                def zxblk_slice(bb, nn):
                    """One 512-col slice of the block-zx pass: h0 of block ib
                    feeds L1 step-block ib, which B-cores execute at position
                    block ib+2 — so accumulate into zx_d rows 2 blocks ahead
                    (A-cores add zeros there)."""
                    ib = bb // DB - 1
                    r0 = (ib + 2) * DB * BL
                    hblk = hblk_live[bb]
                    wk_sb = wihpool.tile([P, KC, 512], bf16, tag="wblkc",
                                         name=f"wc_{bb}_{nn}")
                    nc.scalar.dma_start(
                        wk_sb[:],
                        wblkT.ap().rearrange("(k p) n -> p k n", p=P)[
                            :, :, nn * 512:(nn + 1) * 512])
                    bps = psum_small.tile([P, 512], f32, tag="tps",
                                          name=f"bps_{bb}_{nn}")
                    nc.tensor.matmul(bps[:], onesP[:, :P],
                                     bzblk_sb[:, nn * 512:(nn + 1) * 512],
                                     start=True, stop=False)
                    for k in range(KC):
                        nc.tensor.matmul(bps[:], hblk[:, k, :],
                                         wk_sb[:, k, :],
                                         start=False, stop=(k == KC - 1))
                    zsb = pool.tile([P, 512], bf16, tag="zsb1",
                                    name=f"zs_{bb}_{nn}")
                    nc.vector.tensor_copy(zsb[:], bps[:])
                    nc.gpsimd.dma_start(
                        zx_d.ap().rearrange("t b n -> (t b) n")[
                            r0:r0 + CBL, nn * 512:(nn + 1) * 512],
                        zsb[:], accum_op=TT.add)

                # tick loop: step t runs at tick t; block AGs pipelined.
                # Block ib (steps [ib*DB,(ib+1)*DB)) is flushed by tick
                # (ib+1)*DB, AG'd then; its zx contribution lands in zx_d
                # during ticks [(ib+1)*DB+4, +28]; consumed by the OTHER
                # layer's cores from tick (ib... B-cores simply stall ~2*DB
                # ticks behind A via the Tile-tracked zx_d RAW deps.
                for tau in range(TP_):
                    zps0 = layer_mm(tau)
                    bb = (tau // DB) * DB
                    if bb >= DB:
                        off = tau - bb
                        ib = bb // DB - 1
                        if off == 0:
                            nc.gpsimd.collective_compute(
                                "AllGather",
                                TT.bypass,
                                ins=[cc_in[ib][:].opt()],
                                outs=[cc_out[ib][:].opt()],
                                replica_groups=groups,
                            )
                        if ib < NB:
                            if off == 4:
                                zxblk_hblk(bb)
                            if off >= 6 and (off - 6) % 3 == 0 and (off - 6) // 3 < 8:
                                zxblk_slice(bb, (off - 6) // 3)
                    layer_gates(tau, zps0)
                # final chunk + final AG
                emit_transposes(TP_ - 1)
                flush_chunk(TP_ - 1)
                nc.gpsimd.collective_compute(
                    "AllGather", TT.bypass,
                    ins=[cc_in[NPB - 1][:].opt()],
                    outs=[cc_out[NPB - 1][:].opt()],
                    replica_groups=groups,
                )

        fused_lstm()

        with tc.tile_pool(name="dbgp", bufs=1) as dpool:
            for hh in range(2 * H // P):
                dt_ = dpool.tile([P, CBL], f32, tag="dbg")
                nc.gpsimd.dma_start(dt_[:], cc_out[2].ap()[hh * P:(hh + 1) * P, :])
                nc.sync.dma_start(dbg_ext.ap()[hh * P:(hh + 1) * P, :], dt_[:])
                dt0 = dpool.tile([P, CBL], f32, tag="dbg")
                nc.gpsimd.dma_start(dt0[:], cc_out[0].ap()[hh * P:(hh + 1) * P, :])
                nc.sync.dma_start(dbg0_ext.ap()[hh * P:(hh + 1) * P, :], dt0[:])

        # ---------- Phase G: retrieval + gate MLP on all pair rows ----------
        with tc.tile_pool(name="gkeep", bufs=1) as gkeep:
            h1T_sb = gkeep.tile([P, KC, ROWS], bf16, tag="h1T")
            for ib in range(NB):
                nc.sync.dma_start(
                    h1T_sb[:, :, ib * CBL:(ib + 1) * CBL],
                    cc_out[ib + 2].ap()[H:2 * H, :].rearrange(
                        "(k p) r -> p k r", p=P))

            with tc.tile_pool(name="pg1", bufs=2) as pool, \
                 tc.tile_pool(name="pg1_ps", bufs=2, space="PSUM") as psum_pool, \
                 tc.tile_pool(name="pg1_ps_sm", bufs=1, space="PSUM") as psum_small, \
                 tc.tile_pool(name="pg1_w", bufs=1) as wpool, \
                 tc.tile_pool(name="pg1_keep", bufs=1) as keep:
                segmeanT = keep.tile([P, KC, SB2], f32, tag="segmeanT")
                for k in range(KC):
                    for si in range(Sn):
                        nc.vector.reduce_sum(
                            segmeanT[:, k, si * BL:(si + 1) * BL],
                            h1T_sb[:, k,
                                   si * SEG * BL:(si + 1) * SEG * BL].rearrange(
                                "p (t b) -> p b t", b=BL),
                            axis=AX)
                segbf = keep.tile([P, KC, SB2], bf16, tag="segbf")
                nc.vector.tensor_scalar(segbf.rearrange("p k r -> p (k r)"),
                                        segmeanT.rearrange("p k r -> p (k r)"),
                                        1.0 / SEG, None, TT.mult)

                wkv_sb = wpool.tile([P, KC, H], bf16, tag="wkv")
                nc.sync.dma_start(wkv_sb[:],
                                  wkT.ap().rearrange("(k p) n -> p k n", p=P))
                kbank = keep.tile([SB2, H], bf16, tag="kbank")
                for n2 in range(2):
                    kps = psum_small.tile([SB2, 512], f32, tag="small_ps")
                    for k in range(KC):
                        nc.tensor.matmul(kps[:], segbf[:, k, :],
                                         wkv_sb[:, k, n2 * 512:(n2 + 1) * 512],
                                         start=(k == 0), stop=(k == KC - 1))
                    nc.vector.tensor_copy(kbank[:, n2 * 512:(n2 + 1) * 512], kps[:])
                hlast = keep.tile([P, KC, SB2], bf16, tag="hlast")
                for k in range(KC):
                    nc.vector.tensor_copy(
                        hlast[:, k, :].rearrange("p (t b) -> p t b", b=BL),
                        h1T_sb[:, k, :].rearrange("p (t b) -> p t b", b=BL)[
                            :, SEG - 1::SEG, :])
                wkv_sb2 = wpool.tile([P, KC, H], bf16, tag="wkv")
                nc.sync.dma_start(wkv_sb2[:],
                                  wvT.ap().rearrange("(k p) n -> p k n", p=P))
                vbank = keep.tile([SB2, H], bf16, tag="vbank")
                for n2 in range(2):
                    vps = psum_small.tile([SB2, 512], f32, tag="small_ps")
                    for k in range(KC):
                        nc.tensor.matmul(vps[:], hlast[:, k, :],
                                         wkv_sb2[:, k, n2 * 512:(n2 + 1) * 512],
                                         start=(k == 0), stop=(k == KC - 1))
                    nc.vector.tensor_copy(vbank[:, n2 * 512:(n2 + 1) * 512], vps[:])
                kbankT = keep.tile([P, KC, SB2], bf16, tag="kbankT")
                for k in range(KC):
                    tpsk = psum_small.tile([P, SB2], bf16, tag="tpsb")
                    nc.tensor.transpose(tpsk[:], kbank[:, k * P:(k + 1) * P],
                                        ident[:SB2, :SB2])
                    nc.vector.tensor_copy(kbankT[:, k, :], tpsk[:])

                wq_sb = wpool.tile([P, KC, H], bf16, tag="wq")
                nc.sync.dma_start(wq_sb[:],
                                  wqT.ap().rearrange("(k p) n -> p k n", p=P))

                for m in range(NM):
                    qps = psum_pool.tile([P, H], f32, tag="big_ps")
                    for n2 in range(2):
                        for k in range(KC):
                            nc.tensor.matmul(qps[:, n2 * 512:(n2 + 1) * 512],
                                             h1T_sb[:, k, m * P:(m + 1) * P],
                                             wq_sb[:, k, n2 * 512:(n2 + 1) * 512],
                                             start=(k == 0), stop=(k == KC - 1))
                    q_sb = pool.tile([P, H], bf16, tag="bfA")
                    nc.vector.tensor_copy(q_sb[:], qps[:])
                    qT_sb = pool.tile([P, KC, P], bf16, tag="bfT")
                    for k in range(KC):
                        tps = psum_small.tile([P, P], bf16, tag="tps")
                        nc.tensor.transpose(tps[:], q_sb[:, k * P:(k + 1) * P],
                                            ident[:])
                        nc.vector.tensor_copy(qT_sb[:, k, :], tps[:])
                    sps = psum_small.tile([P, SB2], f32, tag="small_ps")
                    for k in range(KC):
                        nc.tensor.matmul(sps[:], qT_sb[:, k, :], kbankT[:, k, :],
                                         start=(k == 0), stop=(k == KC - 1))
                    amask_sb = pool.tile([P, SB2], f32, tag="amask")
                    nc.sync.dma_start(amask_sb[:],
                                      amask_ext.ap()[m * P:(m + 1) * P, :])
                    mmask_sb = pool.tile([P, SB2], f32, tag="mmask")
                    nc.sync.dma_start(mmask_sb[:],
                                      mmask_ext.ap()[m * P:(m + 1) * P, :])
                    sc = pool.tile([P, SB2], f32, tag="sc")
                    nc.vector.tensor_scalar(sc[:], sps[:], SCALE, None, TT.mult)
                    nc.vector.tensor_tensor(sc[:], sc[:], amask_sb[:], TT.add)
                    smax = pool.tile([P, 1], f32, tag="smax")
                    nc.vector.reduce_max(smax[:], sc[:], axis=AX)
                    nc.vector.tensor_tensor(sc[:], sc[:],
                                            smax[:].to_broadcast((P, SB2)),
                                            TT.subtract)
                    nc.scalar.activation(sc[:], sc[:], AF.Exp)
                    nc.vector.tensor_tensor(sc[:], sc[:], mmask_sb[:], TT.mult)
                    ssum = pool.tile([P, 1], f32, tag="ssum")
                    nc.vector.reduce_sum(ssum[:], sc[:], axis=AX)
                    nc.vector.tensor_scalar(ssum[:], ssum[:], 1e-30, None, TT.add)
                    sinv = pool.tile([P, 1], f32, tag="sinv")
                    nc.vector.reciprocal(sinv[:], ssum[:])
                    attn = pool.tile([P, SB2], bf16, tag="attn")
                    nc.vector.tensor_tensor(attn[:], sc[:],
                                            sinv[:].to_broadcast((P, SB2)), TT.mult)
                    attnT = pool.tile([SB2, P], bf16, tag="attnT")
                    tpsA = psum_small.tile([P, P], bf16, tag="tps")
                    nc.tensor.transpose(tpsA[:SB2, :], attn[:], ident[:])
                    nc.vector.tensor_copy(attnT[:], tpsA[:SB2, :])
                    rps = psum_pool.tile([P, H], f32, tag="big_ps")
                    for n2 in range(2):
                        nc.tensor.matmul(rps[:, n2 * 512:(n2 + 1) * 512], attnT[:],
                                         vbank[:, n2 * 512:(n2 + 1) * 512],
                                         start=True, stop=True)
                    ret_sb = pool.tile([P, H], bf16, tag="bfB")
                    nc.vector.tensor_copy(ret_sb[:], rps[:])
                    nc.sync.dma_start(ret_d.ap()[m * P:(m + 1) * P, :], ret_sb[:])

            # ---- G2: gate MLP + output ----
            with tc.tile_pool(name="pg2", bufs=2) as pool, \
                 tc.tile_pool(name="pg2_ps", bufs=2, space="PSUM") as psum_pool, \
                 tc.tile_pool(name="pg2_ps_sm", bufs=1, space="PSUM") as psum_small, \
                 tc.tile_pool(name="pg2_w", bufs=1) as wpool:
                bias_tiles = {}
                for nm, ap_ in [("bg1", bg1), ("bg2", bg2), ("bo", bo),
                                ("bf", bf_), ("gf", gf), ("bef", bef)]:
                    t_ = wpool.tile([1, H], bf16, tag=nm, name=f"bias_{nm}")
                    nc.sync.dma_start(t_[:], ap_.ap())
                    bias_tiles[nm] = t_
                bg1_sb = bias_tiles["bg1"]; bg2_sb = bias_tiles["bg2"]
                bo_sb = bias_tiles["bo"]; bf_sb = bias_tiles["bf"]
                gf_rep = rep_bias(wpool, psum_pool, bias_tiles["gf"], H, f32, "gf_rep")
                bef_rep = rep_bias(wpool, psum_pool, bias_tiles["bef"], H, f32, "bef_rep")

                wg1h_sb = wpool.tile([P, KC, H], bf16, tag="wg1h")
                nc.sync.dma_start(wg1h_sb[:],
                                  wg1hT.ap().rearrange("(k p) n -> p k n", p=P))
                wg1r_sb = wpool.tile([P, KC, H], bf16, tag="wg1r")
                nc.sync.dma_start(wg1r_sb[:],
                                  wg1rT.ap().rearrange("(k p) n -> p k n", p=P))
                wg2_sb = wpool.tile([P, KC, H], bf16, tag="wg2")
                nc.sync.dma_start(wg2_sb[:],
                                  wg2T.ap().rearrange("(k p) n -> p k n", p=P))
                wo_sb = wpool.tile([P, KC, H], bf16, tag="wo")
                nc.sync.dma_start(wo_sb[:],
                                  woT.ap().rearrange("(k p) n -> p k n", p=P))
                wf_sb = wpool.tile([P, 2 * KC, H], bf16, tag="wf")
                nc.sync.dma_start(wf_sb[:],
                                  wfT.ap().rearrange("(k p) n -> p k n", p=P))

                for m in range(NM):
                    retc = pool.tile([P, H], bf16, tag="retr")
                    nc.sync.dma_start(retc[:], ret_d.ap()[m * P:(m + 1) * P, :])
                    retT = pool.tile([P, KC, P], bf16, tag="retT")
                    for k in range(KC):
                        tps = psum_small.tile([P, P], bf16, tag="tps")
                        nc.tensor.transpose(tps[:], retc[:, k * P:(k + 1) * P],
                                            ident[:])
                        nc.vector.tensor_copy(retT[:, k, :], tps[:])
                    ups = psum_pool.tile([P, H], f32, tag="big_ps")
                    for n2 in range(2):
                        nc.tensor.matmul(ups[:, n2 * 512:(n2 + 1) * 512], onesP[:],
                                         bg1_sb[:, n2 * 512:(n2 + 1) * 512],
                                         start=True, stop=False)
                        for k in range(KC):
                            nc.tensor.matmul(
                                ups[:, n2 * 512:(n2 + 1) * 512],
                                h1T_sb[:, k, m * P:(m + 1) * P],
                                wg1h_sb[:, k, n2 * 512:(n2 + 1) * 512],
                                start=False, stop=False)
                        for k in range(KC):
                            nc.tensor.matmul(
                                ups[:, n2 * 512:(n2 + 1) * 512],
                                retT[:, k, :],
                                wg1r_sb[:, k, n2 * 512:(n2 + 1) * 512],
                                start=False, stop=(k == KC - 1))
                    u1 = pool.tile([P, H], bf16, tag="bfA")
                    nc.vector.tensor_scalar(u1[:], ups[:], 0.0, None, TT.max)
                    u1T = pool.tile([P, KC, P], bf16, tag="bfT")
                    for k in range(KC):
                        tps = psum_small.tile([P, P], bf16, tag="tps")
                        nc.tensor.transpose(tps[:], u1[:, k * P:(k + 1) * P],
                                            ident[:])
                        nc.vector.tensor_copy(u1T[:, k, :], tps[:])
                    gps = psum_pool.tile([P, H], f32, tag="big_ps")
                    for n2 in range(2):
                        nc.tensor.matmul(gps[:, n2 * 512:(n2 + 1) * 512], onesP[:],
                                         bg2_sb[:, n2 * 512:(n2 + 1) * 512],
                                         start=True, stop=False)
                        for k in range(KC):
                            nc.tensor.matmul(gps[:, n2 * 512:(n2 + 1) * 512],
                                             u1T[:, k, :],
                                             wg2_sb[:, k, n2 * 512:(n2 + 1) * 512],
                                             start=False, stop=(k == KC - 1))
                    gate = pool.tile([P, H], f32, tag="gate")
                    nc.scalar.activation(gate[:], gps[:], AF.Sigmoid)
                    # h1 rows via PE transposes of h1T_sb
                    h1r = pool.tile([P, H], bf16, tag="h1r")
                    for k in range(KC):
                        tps = psum_small.tile([P, P], bf16, tag="tps")
                        nc.tensor.transpose(tps[:],
                                            h1T_sb[:, k, m * P:(m + 1) * P],
                                            ident[:])
                        nc.vector.tensor_copy(h1r[:, k * P:(k + 1) * P], tps[:])
                    dmr = pool.tile([P, H], f32, tag="f32tmp")
                    nc.vector.tensor_tensor(dmr[:], h1r[:], retc[:], TT.subtract)
                    mix = pool.tile([P, H], f32, tag="f32tmp")
                    nc.vector.tensor_tensor(mix[:], gate[:], dmr[:], TT.mult)
                    nc.vector.tensor_tensor(mix[:], mix[:], retc[:], TT.add)
                    mixb = pool.tile([P, H], bf16, tag="bfA")
                    nc.vector.tensor_copy(mixb[:], mix[:])
                    mixT = pool.tile([P, KC, P], bf16, tag="bfT")
                    for k in range(KC):
                        tps = psum_small.tile([P, P], bf16, tag="tps")
                        nc.tensor.transpose(tps[:], mixb[:, k * P:(k + 1) * P],
                                            ident[:])
                        nc.vector.tensor_copy(mixT[:, k, :], tps[:])
                    ops_ = psum_pool.tile([P, H], f32, tag="big_ps")
                    for n2 in range(2):
                        nc.tensor.matmul(ops_[:, n2 * 512:(n2 + 1) * 512], onesP[:],
                                         bo_sb[:, n2 * 512:(n2 + 1) * 512],
                                         start=True, stop=False)
                        for k in range(KC):
                            nc.tensor.matmul(ops_[:, n2 * 512:(n2 + 1) * 512],
                                             mixT[:, k, :],
                                             wo_sb[:, k, n2 * 512:(n2 + 1) * 512],
                                             start=False, stop=(k == KC - 1))
                    rzv = pool.tile([P, 1], f32, tag="rzv")
                    nc.sync.dma_start(rzv[:], rz_ext.ap()[m * P:(m + 1) * P, :])
                    rgb = pool.tile([P, H], bf16, tag="bfB")
                    nc.vector.tensor_tensor(rgb[:], ops_[:],
                                            rzv[:].to_broadcast((P, H)), TT.mult)
                    rgT = pool.tile([P, KC, P], bf16, tag="bfT")
                    for k in range(KC):
                        tps = psum_small.tile([P, P], bf16, tag="tps")
                        nc.tensor.transpose(tps[:], rgb[:, k * P:(k + 1) * P],
                                            ident[:])
                        nc.vector.tensor_copy(rgT[:, k, :], tps[:])
                    fps = psum_pool.tile([P, H], f32, tag="big_ps")
                    for n2 in range(2):
                        nc.tensor.matmul(fps[:, n2 * 512:(n2 + 1) * 512], onesP[:],
                                         bf_sb[:, n2 * 512:(n2 + 1) * 512],
                                         start=True, stop=False)
                        for k in range(KC):
                            nc.tensor.matmul(fps[:, n2 * 512:(n2 + 1) * 512],
                                             h1T_sb[:, k, m * P:(m + 1) * P],
                                             wf_sb[:, k, n2 * 512:(n2 + 1) * 512],
                                             start=False, stop=False)
                        for k in range(KC):
                            nc.tensor.matmul(fps[:, n2 * 512:(n2 + 1) * 512],
                                             rgT[:, k, :],
                                             wf_sb[:, KC + k,
                                                   n2 * 512:(n2 + 1) * 512],
                                             start=False, stop=(k == KC - 1))
                    fb = pool.tile([P, H], f32, tag="f32tmp")
                    nc.vector.tensor_copy(fb[:], fps[:])
                    enh = pool.tile([P, H], f32, tag="enh")
                    ln_relu(pool, fb, gf_rep[:], bef_rep[:], enh[:], H)
                    nc.sync.dma_start(out_ext.ap()[m * P:(m + 1) * P, :], enh[:])

        const_pool.__exit__(None, None, None)

    nc.compile()
    return nc


def _prepare_inputs(inputs, Tn=T):
    import ml_dtypes
    gp = _gate_perm()
    f32 = np.float32
    Sn = Tn // SEG
    ROWS = BL * Tn

    def bf(a):
        return np.asarray(a, dtype=ml_dtypes.bfloat16)

    def g(n):
        return np.asarray(inputs[n], f32)

    i2 = np.zeros((P, BL), f32)
    for j in range(NSUB):
        for b in range(BL):
            i2[32 * j + b, b] = 1.0
    rows_t = np.arange(ROWS) // BL
    rows_b = np.arange(ROWS) % BL
    cols_s = np.arange(Sn * BL) // BL
    cols_b = np.arange(Sn * BL) % BL
    valid = (cols_s[None, :] < (rows_t[:, None] // SEG)) & \
            (cols_b[None, :] == rows_b[:, None])
    amask = np.where(valid, 0.0, -1e9).astype(f32)
    mmask = valid.astype(f32)
    rz = (rows_t >= SEG).astype(f32).reshape(ROWS, 1)

    zeros_pre = np.zeros((D, 4 * H), f32)
    zeros_row = np.zeros((1, 4 * H), f32)

    common = {
        "wipT": bf(g("W_ip").T),
        "bip": bf(g("b_ip").reshape(1, -1)), "gip": bf(g("g_ip").reshape(1, -1)),
        "beip": bf(g("be_ip").reshape(1, -1)),
        "wqT": bf(g("Wq").T), "wkT": bf(g("Wk").T), "wvT": bf(g("Wv").T),
        "wg1hT": bf(g("Wg1").T[0:1024] + g("Wg1").T[2048:3072]),
        "wg1rT": bf(g("Wg1").T[1024:2048] - g("Wg1").T[2048:3072]),
        "bg1": bf(g("bg1").reshape(1, -1)),
        "wg2T": bf(g("Wg2").T), "bg2": bf(g("bg2").reshape(1, -1)),
        "woT": bf(g("Wo").T), "bo": bf(g("bo").reshape(1, -1)),
        "wfT": bf(g("Wf").T), "bf": bf(g("bf").reshape(1, -1)),
        "gf": bf(g("g_f").reshape(1, -1)), "bef": bf(g("be_f").reshape(1, -1)),
        "i2blk": bf(i2), "amask": amask, "mmask": mmask, "rz": rz,
    }
    wih0T = g("W_ih0").T[:, gp]
    whh0T = g("W_hh0").T[:, gp]
    bz0 = (g("b_ih0") + g("b_hh0"))[gp].reshape(1, -1)
    wih1T = g("W_ih1").T[:, gp]
    whh1T = g("W_hh1").T[:, gp]
    bz1 = (g("b_ih1") + g("b_hh1"))[gp].reshape(1, -1)

    x = g("x")[:, :Tn, :]
    in_maps = []
    for r in range(N_CORES):
        shard = r % 4
        xs = x[shard * BL:(shard + 1) * BL]
        m = dict(common)
        # rows in (t, b) order, b fastest — matches zx_d/h1T/mask conventions
        m["xT"] = np.ascontiguousarray(
            xs.transpose(1, 0, 2).reshape(Tn * BL, D).T, f32)
        if r < 4:
            m["wpreT"] = bf(wih0T); m["bzpre"] = bf(bz0)
            m["whhT"] = bf(whh0T)
            m["wblkT"] = bf(zeros_pre); m["bzblk"] = bf(zeros_row)
        else:
            m["wpreT"] = bf(zeros_pre); m["bzpre"] = bf(zeros_row)
            m["whhT"] = bf(whh1T)
            m["wblkT"] = bf(wih1T); m["bzblk"] = bf(bz1)
        in_maps.append(m)
    return in_maps


def _run(inputs, Tn=T, **kw):
    from concourse.bass_utils import run_bass_kernel_spmd
    if Tn not in _COMPILED:
        _COMPILED[Tn] = build_bass(Tn)
    nc = _COMPILED[Tn]
    in_maps = _prepare_inputs(inputs, Tn)
    res = run_bass_kernel_spmd(nc, in_maps, core_ids=list(range(N_CORES)), **kw)
    outs = [res.results[r]["out"] for r in range(4)]
    y = np.concatenate([o.reshape(Tn, BL, H).transpose(1, 0, 2) for o in outs],
                       axis=0)
    return np.ascontiguousarray(y, np.float32), res


def kernel(**inputs):
    y, _ = _run(inputs)
    return y


# revision 6
# speedup vs baseline: 1.0061x; 1.0061x over previous
"""Trainium2 Bass kernel for MultiLayerMemoryCachingLSTM — layer-split.

Cores 0-3 run LSTM layer 0, cores 4-7 run layer 1; pair (i, i+4) shares batch
shard [4i, 4i+4). Each core streams only ONE Whh per step (halves the PE-bound
recurrence vs. the dual-layer replicated version). h-blocks of DB steps cross
the pair via ncfw pair AllGathers issued inside the Tile context; zx for the
"other" layer is accumulated into zx_d with per-core zero/real weights so the
SPMD program stays uniform:
  A-cores: phase-B computes zx0 = xp@Wih0 (real); per-block pass adds 0.
  B-cores: phase-B computes zeros; per-block pass adds zx1 = h0_blk@Wih1.
Phase G runs on all pair rows on both cores (identical results); the host
takes A-cores' output.
"""

import math

import numpy as np

B, T, D, H = 16, 256, 1024, 1024
SEG = 16
SCALE = 1.0 / math.sqrt(H)
N_CORES = 8
BL = 4                     # batch rows per pair shard
P = 128
KC = D // P                # 8 contraction chunks
NSUB = 4                   # col groups / hidden subsets
SUB = H // NSUB            # 256 hidden units per subset

_COMPILED = {}


def _gate_perm():
    idx = []
    for j in range(NSUB):
        for g in range(4):
            base = g * H + j * SUB
            idx.extend(range(base, base + SUB))
    return np.array(idx)


def build_bass(Tn=T):
    import concourse.bass as bass
    import concourse.bacc as bacc_mod
    import concourse.mybir as mybir
    import concourse.tile as tile
    from concourse.masks import make_identity

    f32 = mybir.dt.float32
    bf16 = mybir.dt.bfloat16
    AF = mybir.ActivationFunctionType
    TT = mybir.AluOpType
    AX = mybir.AxisListType.X

    Sn = Tn // SEG              # cached segment slots
    ROWS = BL * Tn              # 1024
    NM = ROWS // P              # 8 row chunks
    SB2 = Sn * BL               # bank rows (64)
    DB = 32                     # h-block steps per AllGather
    NB = Tn // DB               # number of real blocks
    NPB = NB + 2                # position blocks (B-cores trail 2 blocks)
    TP_ = NPB * DB              # total positions
    CBL = DB * BL               # block columns (128)

    nc = bacc_mod.Bacc(None, target_bir_lowering=False, num_devices=N_CORES)

    def din(name, shape, dt=bf16):
        return nc.declare_dram_parameter(name, list(shape), dt, isOutput=False)

    xT_ext = din("xT", (D, ROWS), f32)
    wipT = din("wipT", (D, D))
    bip = din("bip", (1, D))
    gip = din("gip", (1, D))
    beip = din("beip", (1, D))
    wpreT = din("wpreT", (D, 4 * H))       # A: Wih0; B: zeros
    bzpre = din("bzpre", (1, 4 * H))       # A: bz0;  B: zeros
    whhT = din("whhT", (H, 4 * H))         # A: Whh0; B: Whh1
    wblkT = din("wblkT", (H, 4 * H))       # A: zeros; B: Wih1
    bzblk = din("bzblk", (1, 4 * H))       # A: zeros; B: bz1
    wqT = din("wqT", (H, H))
    wkT = din("wkT", (H, H))
    wvT = din("wvT", (H, H))
    wg1hT = din("wg1hT", (H, H))
    wg1rT = din("wg1rT", (H, H))
    bg1 = din("bg1", (1, H))
    wg2T = din("wg2T", (H, H))
    bg2 = din("bg2", (1, H))
    woT = din("woT", (H, H))
    bo = din("bo", (1, H))
    wfT = din("wfT", (2 * H, H))
    bf_ = din("bf", (1, H))
    gf = din("gf", (1, H))
    bef = din("bef", (1, H))
    i2blk_ext = din("i2blk", (P, BL))                  # block-diag I4, bf16
    amask_ext = din("amask", (ROWS, SB2), f32)         # additive mask
    mmask_ext = din("mmask", (ROWS, SB2), f32)         # multiplicative mask
    rz_ext = din("rz", (ROWS, 1), f32)                 # 0 for t<SEG else 1

    out_ext = nc.declare_dram_parameter("out", [ROWS, H], f32, isOutput=True)

    zx_d = nc.dram_tensor("zx_d", [TP_, BL, 4 * H], bf16)
    ret_d = nc.dram_tensor("ret_d", [ROWS, H], bf16)
    # pair-AG h blocks: in = my layer's hT block, out = [L0 slab; L1 slab]
    cc_in = [nc.dram_tensor(f"cc_in{ib}", [H, CBL], bf16) for ib in range(NPB)]
    cc_out = [nc.dram_tensor(f"cc_out{ib}", [2 * H, CBL], bf16)
              for ib in range(NPB)]
    groups = [[i, i + 4] for i in range(4)]

    def ln_relu(pool, src, gamma_sb, beta_sb, out_sb, nfree):
        stat = pool.tile([P, 4], f32, tag="ln_stat")
        nc.vector.reduce_sum(stat[:, 0:1], src[:], axis=AX)
        nc.scalar.mul(stat[:, 1:2], stat[:, 0:1], 1.0 / nfree)
        nc.vector.tensor_tensor(src[:], src[:],
                                stat[:, 1:2].to_broadcast((P, nfree)), TT.subtract)
        sq = pool.tile([P, nfree], f32, tag="ln_sq")
        nc.vector.tensor_tensor(sq[:], src[:], src[:], TT.mult)
        nc.vector.reduce_sum(stat[:, 2:3], sq[:], axis=AX)
        nc.vector.tensor_scalar(stat[:, 3:4], stat[:, 2:3], 1.0 / nfree, 1e-5,
                                TT.mult, TT.add)
        nc.scalar.activation(stat[:, 0:1], stat[:, 3:4], AF.Sqrt)
        nc.vector.reciprocal(stat[:, 2:3], stat[:, 0:1])
        nc.vector.tensor_tensor(src[:], src[:],
                                stat[:, 2:3].to_broadcast((P, nfree)), TT.mult)
        nc.vector.tensor_tensor(src[:], src[:], gamma_sb, TT.mult)
        nc.vector.tensor_tensor(src[:], src[:], beta_sb, TT.add)
        nc.vector.tensor_scalar(out_sb, src[:], 0.0, None, TT.max)

    with tile.TileContext(nc) as tc:
        const_pool = tc.tile_pool(name="const", bufs=1)
        const = const_pool.__enter__()
        ident = const.tile([P, P], bf16)
        make_identity(nc, ident)
        i2blk = const.tile([P, BL], bf16)
        nc.sync.dma_start(i2blk[:], i2blk_ext.ap())
        onesP = const.tile([1, P], bf16)
        nc.gpsimd.memset(onesP[:], 1.0)

        def rep_bias(pool_, psum_pool_, bias_row, nfree, dt_out, tag):
            outt = pool_.tile([P, nfree], dt_out, tag=tag, name=f"rep_{tag}")
            for n0 in range(0, nfree, 512):
                w = min(512, nfree - n0)
                ps = psum_pool_.tile([P, 512], f32, tag="tps", name=f"repps_{tag}_{n0}")
                nc.tensor.matmul(ps[:, :w], onesP[:], bias_row[:, n0:n0 + w],
                                 start=True, stop=True)
                nc.vector.tensor_copy(outt[:, n0:n0 + w], ps[:, :w])
            return outt

        # ---------- Phase A: xp = relu(LN(x@Wip+b)) ----------
        with tc.tile_pool(name="pa", bufs=2) as pool, \
             tc.tile_pool(name="pa_ps", bufs=2, space="PSUM") as psum_pool, \
             tc.tile_pool(name="xpT_pool", bufs=1) as xpT_pool, \
             tc.tile_pool(name="pa_w", bufs=1) as wpool:
            xT_sb = wpool.tile([P, KC, ROWS], bf16, tag="xT")
            nc.gpsimd.dma_start(xT_sb[:], xT_ext.ap().rearrange("(k p) r -> p k r", p=P))
            wip_sb = wpool.tile([P, KC, D], bf16, tag="wip")
            nc.sync.dma_start(wip_sb[:], wipT.ap().rearrange("(k p) n -> p k n", p=P))
            bip_sb = wpool.tile([1, D], bf16, tag="bip")
            nc.sync.dma_start(bip_sb[:], bip.ap())
            gip_row = wpool.tile([1, D], bf16, tag="gip")
            nc.sync.dma_start(gip_row[:], gip.ap())
            beip_row = wpool.tile([1, D], bf16, tag="beip")
            nc.sync.dma_start(beip_row[:], beip.ap())
            gip_rep = rep_bias(wpool, psum_pool, gip_row, D, f32, "gip_rep")
            beip_rep = rep_bias(wpool, psum_pool, beip_row, D, f32, "beip_rep")

            xpT_sb = xpT_pool.tile([P, KC, ROWS], bf16, tag="xpT")

            for m in range(NM):
                xp_ps = psum_pool.tile([P, D], f32, tag="xp_ps")
                for n2 in range(2):
                    nc.tensor.matmul(xp_ps[:, n2 * 512:(n2 + 1) * 512], onesP[:],
                                     bip_sb[:, n2 * 512:(n2 + 1) * 512],
                                     start=True, stop=False)
                    for k in range(KC):
                        nc.tensor.matmul(
                            xp_ps[:, n2 * 512:(n2 + 1) * 512],
                            xT_sb[:, k, m * P:(m + 1) * P],
                            wip_sb[:, k, n2 * 512:(n2 + 1) * 512],
                            start=False, stop=(k == KC - 1))
                xpb = pool.tile([P, D], f32, tag="xpb")
                nc.vector.tensor_copy(xpb[:], xp_ps[:])
                xp_sb = pool.tile([P, D], bf16, tag="xp_sb")
                ln_relu(pool, xpb, gip_rep[:], beip_rep[:], xp_sb[:], D)
                for k in range(KC):
                    tps = psum_pool.tile([P, P], bf16, tag="tps")
                    nc.tensor.transpose(tps[:], xp_sb[:, k * P:(k + 1) * P], ident[:])
                    nc.vector.tensor_copy(xpT_sb[:, k, m * P:(m + 1) * P], tps[:])

            # ---------- Phase B: zx_d = xp @ Wpre + bzpre ----------
            bzpre_sb = wpool.tile([1, 4 * H], bf16, tag="bzpre")
            nc.sync.dma_start(bzpre_sb[:], bzpre.ap())
            with tc.tile_pool(name="pb_w", bufs=2) as wpool2:
                for nn in range(8):
                    wk_sb = wpool2.tile([P, KC, 512], bf16, tag="wih_nk")
                    nc.sync.dma_start(
                        wk_sb[:],
                        wpreT.ap().rearrange("(k p) n -> p k n", p=P)[
                            :, :, nn * 512:(nn + 1) * 512])
                    for m in range(NM):
                        zps = psum_pool.tile([P, 512], f32, tag="zps_b")
                        nc.tensor.matmul(zps[:], onesP[:],
                                         bzpre_sb[:, nn * 512:(nn + 1) * 512],
                                         start=True, stop=False)
                        for k in range(KC):
                            nc.tensor.matmul(zps[:], xpT_sb[:, k, m * P:(m + 1) * P],
                                             wk_sb[:, k, :],
                                             start=False, stop=(k == KC - 1))
                        zsb = pool.tile([P, 512], bf16, tag="zsb")
                        nc.vector.tensor_copy(zsb[:], zps[:])
                        nc.sync.dma_start(
                            zx_d.ap().rearrange("t b n -> (t b) n")[
                                m * P:(m + 1) * P, nn * 512:(nn + 1) * 512],
                            zsb[:])

        # zero zx_d tail rows [Tn, TP_) (A-cores' dummy tail + B warmup)
        with tc.tile_pool(name="pz", bufs=1) as pool:
            zt = pool.tile([P, 4 * H], bf16, tag="ztail")
            nc.any.memzero(zt[:])
            ntail = (TP_ - Tn) * BL
            for r0 in range(0, ntail, P):
                nc.sync.dma_start(
                    zx_d.ap().rearrange("t b n -> (t b) n")[
                        Tn * BL + r0:Tn * BL + r0 + P, :], zt[:])

        # ---------- Single-layer recurrence with pair AllGathers ----------
        def fused_lstm():
            CHZ = 2            # zx chunk steps
            CH = 4             # hT/hn chunk steps
            with tc.tile_pool(name="fl", bufs=2) as pool, \
                 tc.tile_pool(name="fl_zx", bufs=2) as zxpool, \
                 tc.tile_pool(name="fl_st", bufs=2) as stpool, \
                 tc.tile_pool(name="fl_ps", bufs=2, space="PSUM") as psum_pool, \
                 tc.tile_pool(name="fl_ps_sm", bufs=2, space="PSUM") as psum_small, \
                 tc.tile_pool(name="fl_w", bufs=1) as wpool, \
                 tc.tile_pool(name="fl_wih", bufs=2) as wihpool, \
                 tc.tile_pool(name="fl_hblk", bufs=2) as hblkpool:
                whh_sb = wpool.tile([P, KC, 4 * H], bf16, tag="whh", name="whh")
                nc.sync.dma_start(whh_sb[:], whhT.ap().rearrange(
                    "(k p) n -> p k n", p=P))
                bzblk_sb = wpool.tile([1, 4 * H], bf16, tag="bzblk")
                nc.sync.dma_start(bzblk_sb[:], bzblk.ap())

                hT0 = wpool.tile([P, 2 * P], bf16, tag="hT0", name="hT0")
                nc.any.memzero(hT0[:])
                c0 = stpool.tile([P, SUB], f32, tag="c", name="c_init")
                nc.any.memzero(c0[:])
                st = dict(hT0=hT0, c=c0, zx=None, zxnext=None,
                          hTbuf=None, hTprev=None, hnbuf=None, hnprev=None)

                def emit_transposes(t):
                    s = st
                    sl = t % CH
                    hn = s["hnbuf"]
                    for half in range(2):
                        tps = psum_small.tile([P, P], bf16, tag="tps",
                                              name=f"tp_{t}_{half}")
                        nc.tensor.transpose(
                            tps[:], hn[:, sl, half * P:(half + 1) * P], ident[:])
                        nc.vector.tensor_copy(
                            s["hTbuf"][:, sl, half * P:(half + 1) * P], tps[:])

                def flush_chunk(t_last):
                    """DMA the completed hT chunk ending at t_last into its
                    cc_in block tensor (columns relative to the block)."""
                    s = st
                    t0 = t_last - CH + 1
                    ib = t0 // DB
                    c0_ = (t0 - ib * DB) * BL
                    for half in range(2):
                        for j in range(NSUB):
                            off = 128 * half + 32 * j
                            nc.sync.dma_start(
                                cc_in[ib].ap()[256 * j + 128 * half:
                                               256 * j + 128 * (half + 1),
                                               c0_:c0_ + CH * BL].rearrange(
                                    "u (t b) -> u t b", b=BL),
                                s["hTbuf"][:, :, off:off + BL])

                def load_zx(t0):
                    zx = zxpool.tile([P, CHZ, H], bf16, tag="zx",
                                     name=f"zx_{t0}")
                    for j in range(NSUB):
                        nc.sync.dma_start(
                            zx[32 * j:32 * j + BL, :, :],
                            zx_d.ap()[t0:t0 + CHZ, :,
                                      j * H:(j + 1) * H].rearrange(
                                "t b n -> b t n"))
                    return zx

                def ensure_zx(t):
                    s = st
                    if t % CHZ == 0:
                        s["zx"] = load_zx(t) if t == 0 else s["zxnext"]
                        s["zxnext"] = load_zx(t + CHZ) if t + CHZ < TP_ else None

                def mm_whh(t):
                    """Whh-only z accumulation for step t (zx is added on the
                    DVE in layer_gates — no PE inject matmuls)."""
                    s = st
                    sl = t % CH
                    zps = psum_pool.tile([P, H], f32, tag="zps",
                                         name=f"zps_{t}")
                    for k in range(KC):
                        half, jj = k % 2, k // 2
                        off = 128 * half + 32 * jj
                        if t == 0:
                            lhs = s["hT0"][:, off:off + BL]
                        elif sl == 0:
                            lhs = s["hTprev"][:, CH - 1, off:off + BL]
                        else:
                            lhs = s["hTbuf"][:, sl - 1, off:off + BL]
                        for n2 in range(2):
                            for j in range(NSUB):
                                nc.tensor.matmul(
                                    zps[32 * j:32 * j + BL,
                                        n2 * 512:(n2 + 1) * 512],
                                    lhs,
                                    whh_sb[:, k, j * H + n2 * 512:
                                           j * H + (n2 + 1) * 512],
                                    start=(k == 0), stop=(k == KC - 1),
                                    tile_position=(0, 32 * j))
                    return zps

                def layer_gates(t, zps):
                    """z = zps + zx via DVE (pipelined per gate slice), then
                    split activations so the c-chain starts early."""
                    s = st
                    sl = t % CH
                    slz = t % CHZ
                    zadd = pool.tile([P, H], f32, tag="zadd", name=f"za_{t}")
                    sig = pool.tile([P, H], bf16, tag="sig", name=f"sg_{t}")
                    tg = pool.tile([P, SUB], bf16, tag="tg", name=f"tg_{t}")
                    nc.vector.tensor_tensor(zadd[:, 2 * SUB:3 * SUB],
                                            zps[:, 2 * SUB:3 * SUB],
                                            s["zx"][:, slz, 2 * SUB:3 * SUB],
                                            TT.add)
                    nc.scalar.activation(tg[:], zadd[:, 2 * SUB:3 * SUB], AF.Tanh)
                    nc.vector.tensor_tensor(zadd[:, 0:2 * SUB],
                                            zps[:, 0:2 * SUB],
                                            s["zx"][:, slz, 0:2 * SUB], TT.add)
                    nc.scalar.activation(sig[:, 0:2 * SUB], zadd[:, 0:2 * SUB],
                                         AF.Sigmoid)
                    nc.vector.tensor_tensor(zadd[:, 3 * SUB:4 * SUB],
                                            zps[:, 3 * SUB:4 * SUB],
                                            s["zx"][:, slz, 3 * SUB:4 * SUB],
                                            TT.add)
                    nc.scalar.activation(sig[:, 3 * SUB:4 * SUB],
                                         zadd[:, 3 * SUB:4 * SUB], AF.Sigmoid)
                    cf = pool.tile([P, SUB], f32, tag="cf", name=f"cf_{t}")
                    nc.vector.tensor_tensor(cf[:], sig[:, SUB:2 * SUB],
                                            s["c"][:], TT.mult)
                    ig = pool.tile([P, SUB], f32, tag="ig", name=f"ig_{t}")
                    nc.vector.tensor_tensor(ig[:], sig[:, 0:SUB], tg[:], TT.mult)
                    c_new = stpool.tile([P, SUB], f32, tag="c", name=f"c_{t}")
                    nc.vector.tensor_tensor(c_new[:], cf[:], ig[:], TT.add)
                    s["c"] = c_new
                    tct = pool.tile([P, SUB], bf16, tag="tct", name=f"tc_{t}")
                    nc.scalar.activation(tct[:], c_new[:], AF.Tanh)
                    nc.vector.tensor_tensor(s["hnbuf"][:, sl, :],
                                            sig[:, 3 * SUB:4 * SUB], tct[:],
                                            TT.mult)

                def rotate_chunks(t):
                    """Prepare hTbuf/hnbuf for the chunk starting at step t."""
                    s = st
                    s["hTprev"] = s["hTbuf"]
                    s["hTbuf"] = stpool.tile([P, CH, 2 * P], bf16,
                                             tag="hTb", name=f"hTb_{t}")
                    s["hnprev"] = s["hnbuf"]
                    s["hnbuf"] = stpool.tile([P, CH, SUB], bf16,
                                             tag="hnb", name=f"hnb_{t}")

                hblk_live = {}

                wk_live = {}

                def load_wk(bb, nn):
                    wk_sb = wihpool.tile([P, KC, 512], bf16, tag="wblkc",
                                         name=f"wc_{bb}_{nn}")
                    nc.scalar.dma_start(
                        wk_sb[:],
                        wblkT.ap().rearrange("(k p) n -> p k n", p=P)[
                            :, :, nn * 512:(nn + 1) * 512])
                    wk_live[(bb, nn)] = wk_sb

                def zxblk_hblk(bb):
                    """Load the L0 slab of block bb-DB..bb from cc_out."""
                    ib = bb // DB - 1
                    hblk = hblkpool.tile([P, KC, CBL], bf16, tag="hblk",
                                         name=f"hblk_{bb}")
                    nc.scalar.dma_start(
                        hblk[:], cc_out[ib].ap()[0:H, :].rearrange(
                            "(k p) r -> p k r", p=P))
                    hblk_live[bb] = hblk
                    load_wk(bb, 0)

                bps_live = {}

                def zxblk_part(bb, nn, part):
                    """One third of a 512-col slice of the block-zx pass
                    (bias+k0-2 / k3-5 / k6-7+store), spread across ticks as
                    PE filler that is independent of the recurrence chain."""
                    ib = bb // DB - 1
                    r0 = (ib + 2) * DB * BL
                    hblk = hblk_live[bb]
                    if part == 0:
                        if nn + 1 < 8:
                            load_wk(bb, nn + 1)
                        wk_sb = wk_live[(bb, nn)]
                        bps = psum_small.tile([P, 512], f32, tag="bps",
                                              name=f"bps_{bb}_{nn}")
                        bps_live[(bb, nn)] = (bps, wk_sb)
                        nc.tensor.matmul(bps[:], onesP[:, :P],
                                         bzblk_sb[:, nn * 512:(nn + 1) * 512],
                                         start=True, stop=False)
                        for k in range(0, 3):
                            nc.tensor.matmul(bps[:], hblk[:, k, :],
                                             wk_sb[:, k, :],
                                             start=False, stop=False)
                    elif part == 1:
                        bps, wk_sb = bps_live[(bb, nn)]
                        for k in range(3, 6):
                            nc.tensor.matmul(bps[:], hblk[:, k, :],
                                             wk_sb[:, k, :],
                                             start=False, stop=False)
                    else:
                        bps, wk_sb = bps_live[(bb, nn)]
                        for k in range(6, KC):
                            nc.tensor.matmul(bps[:], hblk[:, k, :],
                                             wk_sb[:, k, :],
                                             start=False, stop=(k == KC - 1))
                        zsb = pool.tile([P, 512], bf16, tag="zsb1",
                                        name=f"zs_{bb}_{nn}")
                        nc.vector.tensor_copy(zsb[:], bps[:])
                        nc.gpsimd.dma_start(
                            zx_d.ap().rearrange("t b n -> (t b) n")[
                                r0:r0 + CBL, nn * 512:(nn + 1) * 512],
                            zsb[:], accum_op=TT.add)

                # Tick order: whh(t) -> zx1-slice part (filler) ->
                # inject(t+1) (filler) -> transposes(t) [stall on gates(t)
                # covered by the fillers] -> flush/AG at chunk/block ends.
                for tau in range(TP_):
                    ensure_zx(tau)
                    if tau % CH == 0:
                        rotate_chunks(tau)
                    zps = mm_whh(tau)
                    bb = (tau // DB) * DB
                    ib = bb // DB - 1
                    if bb >= DB and ib < NB:
                        off = tau - bb
                        if off == 3:
                            zxblk_hblk(bb)
                        if 6 <= off < 30:
                            nn, part = (off - 6) // 3, (off - 6) % 3
                            zxblk_part(bb, nn, part)
                    layer_gates(tau, zps)
                    emit_transposes(tau)
                    if tau % CH == CH - 1:
                        flush_chunk(tau)
                    if tau % DB == DB - 1 and tau > 0:
                        jb = tau // DB
                        nc.gpsimd.collective_compute(
                            "AllGather",
                            TT.bypass,
                            ins=[cc_in[jb][:].opt()],
                            outs=[cc_out[jb][:].opt()],
                            replica_groups=groups,
                        )

        fused_lstm()

        # ---------- Phase G: retrieval + gate MLP on all pair rows ----------
        with tc.tile_pool(name="gkeep", bufs=1) as gkeep:
            h1T_sb = gkeep.tile([P, KC, ROWS], bf16, tag="h1T")
            for ib in range(NB):
                nc.sync.dma_start(
                    h1T_sb[:, :, ib * CBL:(ib + 1) * CBL],
                    cc_out[ib + 2].ap()[H:2 * H, :].rearrange(
                        "(k p) r -> p k r", p=P))

            gw = tc.tile_pool(name="gw", bufs=1)
            gwp = gw.__enter__()
            wg1h_sb = gwp.tile([P, KC, H], bf16, tag="wg1h")
            nc.gpsimd.dma_start(wg1h_sb[:],
                                wg1hT.ap().rearrange("(k p) n -> p k n", p=P))
            wg1r_sb = gwp.tile([P, KC, H], bf16, tag="wg1r")
            nc.gpsimd.dma_start(wg1r_sb[:],
                                wg1rT.ap().rearrange("(k p) n -> p k n", p=P))
            wg2_sb = gwp.tile([P, KC, H], bf16, tag="wg2")
            nc.gpsimd.dma_start(wg2_sb[:],
                                wg2T.ap().rearrange("(k p) n -> p k n", p=P))
            wo_sb = gwp.tile([P, KC, H], bf16, tag="wo")
            nc.gpsimd.dma_start(wo_sb[:],
                                woT.ap().rearrange("(k p) n -> p k n", p=P))
            wf_sb = gwp.tile([P, 2 * KC, H], bf16, tag="wf")
            nc.gpsimd.dma_start(wf_sb[:],
                                wfT.ap().rearrange("(k p) n -> p k n", p=P))

            with tc.tile_pool(name="pg1", bufs=2) as pool, \
                 tc.tile_pool(name="pg1_ps", bufs=2, space="PSUM") as psum_pool, \
                 tc.tile_pool(name="pg1_ps_sm", bufs=1, space="PSUM") as psum_small, \
                 tc.tile_pool(name="pg1_w", bufs=1) as wpool, \
                 tc.tile_pool(name="pg1_keep", bufs=1) as keep:
                segmeanT = keep.tile([P, KC, SB2], f32, tag="segmeanT")
                for k in range(KC):
                    for si in range(Sn):
                        nc.vector.reduce_sum(
                            segmeanT[:, k, si * BL:(si + 1) * BL],
                            h1T_sb[:, k,
                                   si * SEG * BL:(si + 1) * SEG * BL].rearrange(
                                "p (t b) -> p b t", b=BL),
                            axis=AX)
                segbf = keep.tile([P, KC, SB2], bf16, tag="segbf")
                nc.vector.tensor_scalar(segbf.rearrange("p k r -> p (k r)"),
                                        segmeanT.rearrange("p k r -> p (k r)"),
                                        1.0 / SEG, None, TT.mult)

                wkv_sb = wpool.tile([P, KC, H], bf16, tag="wkv")
                nc.sync.dma_start(wkv_sb[:],
                                  wkT.ap().rearrange("(k p) n -> p k n", p=P))
                kbank = keep.tile([SB2, H], bf16, tag="kbank")
                for n2 in range(2):
                    kps = psum_small.tile([SB2, 512], f32, tag="small_ps")
                    for k in range(KC):
                        nc.tensor.matmul(kps[:], segbf[:, k, :],
                                         wkv_sb[:, k, n2 * 512:(n2 + 1) * 512],
                                         start=(k == 0), stop=(k == KC - 1))
                    nc.vector.tensor_copy(kbank[:, n2 * 512:(n2 + 1) * 512], kps[:])
                hlast = keep.tile([P, KC, SB2], bf16, tag="hlast")
                for k in range(KC):
                    nc.vector.tensor_copy(
                        hlast[:, k, :].rearrange("p (t b) -> p t b", b=BL),
                        h1T_sb[:, k, :].rearrange("p (t b) -> p t b", b=BL)[
                            :, SEG - 1::SEG, :])
                wkv_sb2 = wpool.tile([P, KC, H], bf16, tag="wkv")
                nc.sync.dma_start(wkv_sb2[:],
                                  wvT.ap().rearrange("(k p) n -> p k n", p=P))
                vbank = keep.tile([SB2, H], bf16, tag="vbank")
                for n2 in range(2):
                    vps = psum_small.tile([SB2, 512], f32, tag="small_ps")
                    for k in range(KC):
                        nc.tensor.matmul(vps[:], hlast[:, k, :],
                                         wkv_sb2[:, k, n2 * 512:(n2 + 1) * 512],
                                         start=(k == 0), stop=(k == KC - 1))
                    nc.vector.tensor_copy(vbank[:, n2 * 512:(n2 + 1) * 512], vps[:])
                kbankT = keep.tile([P, KC, SB2], bf16, tag="kbankT")
                for k in range(KC):
                    tpsk = psum_small.tile([P, SB2], bf16, tag="tpsb")
                    nc.tensor.transpose(tpsk[:], kbank[:, k * P:(k + 1) * P],
                                        ident[:SB2, :SB2])
                    nc.vector.tensor_copy(kbankT[:, k, :], tpsk[:])

                wq_sb = wpool.tile([P, KC, H], bf16, tag="wq")
                nc.sync.dma_start(wq_sb[:],
                                  wqT.ap().rearrange("(k p) n -> p k n", p=P))

                for m in range(NM):
                    qps = psum_pool.tile([P, H], f32, tag="big_ps")
                    for n2 in range(2):
                        for k in range(KC):
                            nc.tensor.matmul(qps[:, n2 * 512:(n2 + 1) * 512],
                                             h1T_sb[:, k, m * P:(m + 1) * P],
                                             wq_sb[:, k, n2 * 512:(n2 + 1) * 512],
                                             start=(k == 0), stop=(k == KC - 1))
                    q_sb = pool.tile([P, H], bf16, tag="bfA")
                    nc.vector.tensor_copy(q_sb[:], qps[:])
                    qT_sb = pool.tile([P, KC, P], bf16, tag="bfT")
                    for k in range(KC):
                        tps = psum_small.tile([P, P], bf16, tag="tps")
                        nc.tensor.transpose(tps[:], q_sb[:, k * P:(k + 1) * P],
                                            ident[:])
                        nc.vector.tensor_copy(qT_sb[:, k, :], tps[:])
                    sps = psum_small.tile([P, SB2], f32, tag="small_ps")
                    for k in range(KC):
                        nc.tensor.matmul(sps[:], qT_sb[:, k, :], kbankT[:, k, :],
                                         start=(k == 0), stop=(k == KC - 1))
                    amask_sb = pool.tile([P, SB2], f32, tag="amask")
                    nc.sync.dma_start(amask_sb[:],
                                      amask_ext.ap()[m * P:(m + 1) * P, :])
                    mmask_sb = pool.tile([P, SB2], f32, tag="mmask")
                    nc.sync.dma_start(mmask_sb[:],
                                      mmask_ext.ap()[m * P:(m + 1) * P, :])
                    sc = pool.tile([P, SB2], f32, tag="sc")
                    nc.vector.tensor_scalar(sc[:], sps[:], SCALE, None, TT.mult)
                    nc.vector.tensor_tensor(sc[:], sc[:], amask_sb[:], TT.add)
                    smax = pool.tile([P, 1], f32, tag="smax")
                    nc.vector.reduce_max(smax[:], sc[:], axis=AX)
                    nc.vector.tensor_tensor(sc[:], sc[:],
                                            smax[:].to_broadcast((P, SB2)),
                                            TT.subtract)
                    nc.scalar.activation(sc[:], sc[:], AF.Exp)
                    nc.vector.tensor_tensor(sc[:], sc[:], mmask_sb[:], TT.mult)
                    ssum = pool.tile([P, 1], f32, tag="ssum")
                    nc.vector.reduce_sum(ssum[:], sc[:], axis=AX)
                    nc.vector.tensor_scalar(ssum[:], ssum[:], 1e-30, None, TT.add)
                    sinv = pool.tile([P, 1], f32, tag="sinv")
                    nc.vector.reciprocal(sinv[:], ssum[:])
                    attn = pool.tile([P, SB2], bf16, tag="attn")
                    nc.vector.tensor_tensor(attn[:], sc[:],
                                            sinv[:].to_broadcast((P, SB2)), TT.mult)
                    attnT = pool.tile([SB2, P], bf16, tag="attnT")
                    tpsA = psum_small.tile([P, P], bf16, tag="tps")
                    nc.tensor.transpose(tpsA[:SB2, :], attn[:], ident[:])
                    nc.vector.tensor_copy(attnT[:], tpsA[:SB2, :])
                    rps = psum_pool.tile([P, H], f32, tag="big_ps")
                    for n2 in range(2):
                        nc.tensor.matmul(rps[:, n2 * 512:(n2 + 1) * 512], attnT[:],
                                         vbank[:, n2 * 512:(n2 + 1) * 512],
                                         start=True, stop=True)
                    ret_sb = pool.tile([P, H], bf16, tag="bfB")
                    nc.vector.tensor_copy(ret_sb[:], rps[:])
                    nc.sync.dma_start(ret_d.ap()[m * P:(m + 1) * P, :], ret_sb[:])

            # ---- G2: gate MLP + output ----
            with tc.tile_pool(name="pg2", bufs=2) as pool, \
                 tc.tile_pool(name="pg2_ps", bufs=2, space="PSUM") as psum_pool, \
                 tc.tile_pool(name="pg2_ps_sm", bufs=1, space="PSUM") as psum_small, \
                 tc.tile_pool(name="pg2_w", bufs=1) as wpool:
                bias_tiles = {}
                for nm, ap_ in [("bg1", bg1), ("bg2", bg2), ("bo", bo),
                                ("bf", bf_), ("gf", gf), ("bef", bef)]:
                    t_ = wpool.tile([1, H], bf16, tag=nm, name=f"bias_{nm}")
                    nc.sync.dma_start(t_[:], ap_.ap())
                    bias_tiles[nm] = t_
                bg1_sb = bias_tiles["bg1"]; bg2_sb = bias_tiles["bg2"]
                bo_sb = bias_tiles["bo"]; bf_sb = bias_tiles["bf"]
                gf_rep = rep_bias(wpool, psum_pool, bias_tiles["gf"], H, f32, "gf_rep")
                bef_rep = rep_bias(wpool, psum_pool, bias_tiles["bef"], H, f32, "bef_rep")

                for m in range(NM):
                    retc = pool.tile([P, H], bf16, tag="retr")
                    nc.sync.dma_start(retc[:], ret_d.ap()[m * P:(m + 1) * P, :])
                    retT = pool.tile([P, KC, P], bf16, tag="retT")
                    for k in range(KC):
                        tps = psum_small.tile([P, P], bf16, tag="tps")
                        nc.tensor.transpose(tps[:], retc[:, k * P:(k + 1) * P],
                                            ident[:])
                        nc.vector.tensor_copy(retT[:, k, :], tps[:])
                    ups = psum_pool.tile([P, H], f32, tag="big_ps")
                    for n2 in range(2):
                        nc.tensor.matmul(ups[:, n2 * 512:(n2 + 1) * 512], onesP[:],
                                         bg1_sb[:, n2 * 512:(n2 + 1) * 512],
                                         start=True, stop=False)
                        for k in range(KC):
                            nc.tensor.matmul(
                                ups[:, n2 * 512:(n2 + 1) * 512],
                                h1T_sb[:, k, m * P:(m + 1) * P],
                                wg1h_sb[:, k, n2 * 512:(n2 + 1) * 512],
                                start=False, stop=False)
                        for k in range(KC):
                            nc.tensor.matmul(
                                ups[:, n2 * 512:(n2 + 1) * 512],
                                retT[:, k, :],
                                wg1r_sb[:, k, n2 * 512:(n2 + 1) * 512],
                                start=False, stop=(k == KC - 1))
                    u1 = pool.tile([P, H], bf16, tag="bfA")
                    nc.vector.tensor_scalar(u1[:], ups[:], 0.0, None, TT.max)
                    u1T = pool.tile([P, KC, P], bf16, tag="bfT")
                    for k in range(KC):
                        tps = psum_small.tile([P, P], bf16, tag="tps")
                        nc.tensor.transpose(tps[:], u1[:, k * P:(k + 1) * P],
                                            ident[:])
                        nc.vector.tensor_copy(u1T[:, k, :], tps[:])
                    gps = psum_pool.tile([P, H], f32, tag="big_ps")
                    for n2 in range(2):
                        nc.tensor.matmul(gps[:, n2 * 512:(n2 + 1) * 512], onesP[:],
                                         bg2_sb[:, n2 * 512:(n2 + 1) * 512],
                                         start=True, stop=False)
                        for k in range(KC):
                            nc.tensor.matmul(gps[:, n2 * 512:(n2 + 1) * 512],
                                             u1T[:, k, :],
                                             wg2_sb[:, k, n2 * 512:(n2 + 1) * 512],
                                             start=False, stop=(k == KC - 1))
                    gate = pool.tile([P, H], f32, tag="gate")
                    nc.scalar.activation(gate[:], gps[:], AF.Sigmoid)
                    # h1 rows via PE transposes of h1T_sb
                    h1r = pool.tile([P, H], bf16, tag="h1r")
                    for k in range(KC):
                        tps = psum_small.tile([P, P], bf16, tag="tps")
                        nc.tensor.transpose(tps[:],
                                            h1T_sb[:, k, m * P:(m + 1) * P],
                                            ident[:])
                        nc.vector.tensor_copy(h1r[:, k * P:(k + 1) * P], tps[:])
                    dmr = pool.tile([P, H], f32, tag="f32tmp")
                    nc.vector.tensor_tensor(dmr[:], h1r[:], retc[:], TT.subtract)
                    mix = pool.tile([P, H], f32, tag="f32tmp")
                    nc.vector.tensor_tensor(mix[:], gate[:], dmr[:], TT.mult)
                    nc.vector.tensor_tensor(mix[:], mix[:], retc[:], TT.add)
                    mixb = pool.tile([P, H], bf16, tag="bfA")
                    nc.vector.tensor_copy(mixb[:], mix[:])
                    mixT = pool.tile([P, KC, P], bf16, tag="bfT")
                    for k in range(KC):
                        tps = psum_small.tile([P, P], bf16, tag="tps")
                        nc.tensor.transpose(tps[:], mixb[:, k * P:(k + 1) * P],
                                            ident[:])
                        nc.vector.tensor_copy(mixT[:, k, :], tps[:])
                    ops_ = psum_pool.tile([P, H], f32, tag="big_ps")
                    for n2 in range(2):
                        nc.tensor.matmul(ops_[:, n2 * 512:(n2 + 1) * 512], onesP[:],
                                         bo_sb[:, n2 * 512:(n2 + 1) * 512],
                                         start=True, stop=False)
                        for k in range(KC):
                            nc.tensor.matmul(ops_[:, n2 * 512:(n2 + 1) * 512],
                                             mixT[:, k, :],
                                             wo_sb[:, k, n2 * 512:(n2 + 1) * 512],
                                             start=False, stop=(k == KC - 1))
                    rzv = pool.tile([P, 1], f32, tag="rzv")
                    nc.sync.dma_start(rzv[:], rz_ext.ap()[m * P:(m + 1) * P, :])
                    rgb = pool.tile([P, H], bf16, tag="bfB")
                    nc.vector.tensor_tensor(rgb[:], ops_[:],
                                            rzv[:].to_broadcast((P, H)), TT.mult)
                    rgT = pool.tile([P, KC, P], bf16, tag="bfT")
                    for k in range(KC):
                        tps = psum_small.tile([P, P], bf16, tag="tps")
                        nc.tensor.transpose(tps[:], rgb[:, k * P:(k + 1) * P],
                                            ident[:])
                        nc.vector.tensor_copy(rgT[:, k, :], tps[:])
                    fps = psum_pool.tile([P, H], f32, tag="big_ps")
                    for n2 in range(2):
                        nc.tensor.matmul(fps[:, n2 * 512:(n2 + 1) * 512], onesP[:],
                                         bf_sb[:, n2 * 512:(n2 + 1) * 512],
                                         start=True, stop=False)
                        for k in range(KC):
                            nc.tensor.matmul(fps[:, n2 * 512:(n2 + 1) * 512],
                                             h1T_sb[:, k, m * P:(m + 1) * P],
                                             wf_sb[:, k, n2 * 512:(n2 + 1) * 512],
                                             start=False, stop=False)
                        for k in range(KC):
                            nc.tensor.matmul(fps[:, n2 * 512:(n2 + 1) * 512],
                                             rgT[:, k, :],
                                             wf_sb[:, KC + k,
                                                   n2 * 512:(n2 + 1) * 512],
                                             start=False, stop=(k == KC - 1))
                    fb = pool.tile([P, H], f32, tag="f32tmp")
                    nc.vector.tensor_copy(fb[:], fps[:])
                    enh = pool.tile([P, H], f32, tag="enh")
                    ln_relu(pool, fb, gf_rep[:], bef_rep[:], enh[:], H)
                    nc.sync.dma_start(out_ext.ap()[m * P:(m + 1) * P, :], enh[:])

            gw.__exit__(None, None, None)

        const_pool.__exit__(None, None, None)

    nc.compile()
    return nc


def _prepare_inputs(inputs, Tn=T):
    import ml_dtypes
    gp = _gate_perm()
    f32 = np.float32
    Sn = Tn // SEG
    ROWS = BL * Tn

    def bf(a):
        return np.asarray(a, dtype=ml_dtypes.bfloat16)

    def g(n):
        return np.asarray(inputs[n], f32)

    i2 = np.zeros((P, BL), f32)
    for j in range(NSUB):
        for b in range(BL):
            i2[32 * j + b, b] = 1.0
    rows_t = np.arange(ROWS) // BL
    rows_b = np.arange(ROWS) % BL
    cols_s = np.arange(Sn * BL) // BL
    cols_b = np.arange(Sn * BL) % BL
    valid = (cols_s[None, :] < (rows_t[:, None] // SEG)) & \
            (cols_b[None, :] == rows_b[:, None])
    amask = np.where(valid, 0.0, -1e9).astype(f32)
    mmask = valid.astype(f32)
    rz = (rows_t >= SEG).astype(f32).reshape(ROWS, 1)

    zeros_pre = np.zeros((D, 4 * H), f32)
    zeros_row = np.zeros((1, 4 * H), f32)

    common = {
        "wipT": bf(g("W_ip").T),
        "bip": bf(g("b_ip").reshape(1, -1)), "gip": bf(g("g_ip").reshape(1, -1)),
        "beip": bf(g("be_ip").reshape(1, -1)),
        "wqT": bf(g("Wq").T), "wkT": bf(g("Wk").T), "wvT": bf(g("Wv").T),
        "wg1hT": bf(g("Wg1").T[0:1024] + g("Wg1").T[2048:3072]),
        "wg1rT": bf(g("Wg1").T[1024:2048] - g("Wg1").T[2048:3072]),
        "bg1": bf(g("bg1").reshape(1, -1)),
        "wg2T": bf(g("Wg2").T), "bg2": bf(g("bg2").reshape(1, -1)),
        "woT": bf(g("Wo").T), "bo": bf(g("bo").reshape(1, -1)),
        "wfT": bf(g("Wf").T), "bf": bf(g("bf").reshape(1, -1)),
        "gf": bf(g("g_f").reshape(1, -1)), "bef": bf(g("be_f").reshape(1, -1)),
        "i2blk": bf(i2), "amask": amask, "mmask": mmask, "rz": rz,
    }
    wih0T = g("W_ih0").T[:, gp]
    whh0T = g("W_hh0").T[:, gp]
    bz0 = (g("b_ih0") + g("b_hh0"))[gp].reshape(1, -1)
    wih1T = g("W_ih1").T[:, gp]
    whh1T = g("W_hh1").T[:, gp]
    bz1 = (g("b_ih1") + g("b_hh1"))[gp].reshape(1, -1)

    x = g("x")[:, :Tn, :]
    in_maps = []
    for r in range(N_CORES):
        shard = r % 4
        xs = x[shard * BL:(shard + 1) * BL]
        m = dict(common)
        # rows in (t, b) order, b fastest — matches zx_d/h1T/mask conventions
        m["xT"] = np.ascontiguousarray(
            xs.transpose(1, 0, 2).reshape(Tn * BL, D).T, f32)
        if r < 4:
            m["wpreT"] = bf(wih0T); m["bzpre"] = bf(bz0)
            m["whhT"] = bf(whh0T)
            m["wblkT"] = bf(zeros_pre); m["bzblk"] = bf(zeros_row)
        else:
            m["wpreT"] = bf(zeros_pre); m["bzpre"] = bf(zeros_row)
            m["whhT"] = bf(whh1T)
            m["wblkT"] = bf(wih1T); m["bzblk"] = bf(bz1)
        in_maps.append(m)
    return in_maps


def _run(inputs, Tn=T, **kw):
    from concourse.bass_utils import run_bass_kernel_spmd
    if Tn not in _COMPILED:
        _COMPILED[Tn] = build_bass(Tn)
    nc = _COMPILED[Tn]
    in_maps = _prepare_inputs(inputs, Tn)
    res = run_bass_kernel_spmd(nc, in_maps, core_ids=list(range(N_CORES)), **kw)
    outs = [res.results[r]["out"] for r in range(4)]
    y = np.concatenate([o.reshape(Tn, BL, H).transpose(1, 0, 2) for o in outs],
                       axis=0)
    return np.ascontiguousarray(y, np.float32), res


def kernel(**inputs):
    y, _ = _run(inputs)
    return y
